# revision 1
# baseline (speedup 1.0000x reference)
"""Graphormer attention Trainium2 kernel.

Problem: B=4, N=1024, D=256, H=8 heads (Dh=32), binned relative bias
  idx = clip(int(z/5*16), 0, 15);  scores = QK^T*scale + z_emb[idx]
  softmax over keys (key_mask additive -inf), out = attn @ V -> out_proj.

Sharding: 8 cores <- (batch b, query-row half). Each core computes rows
[half*512, half*512+512) of batch b for all 8 heads. No collectives;
host slices inputs / concatenates outputs.

Device algorithm (transposed layout, keys on partitions):
  S^T[k, q] accumulated in PSUM:
     QK part:  matmul(lhsT=K^T_h [32d,128k], rhs=Q^T_h [32d,512q])  (fp32)
   + bias part: 15 cumulative threshold masks M_t[k,q] = (z*3.2 >= t)
     (fp16, exact 0/1) accumulated via scaled-identity matmuls
     lhsT=((z_emb[t,h]-z_emb[t-1,h])/scale * I128), rhs=M_t
     (cumulative masks == exact 16-bin staircase LUT).
  E^T = exp(S^T*scale + (z_emb[0,h] + keymask*-1e30))  ScalarE, fp16 out
  NUM^T[d|Z, q] += matmul(lhsT=V_aug[128k, 33], rhs=E^T); V col 32 = ones
     -> NUM row 32 = softmax denominator Z (deferred normalization).
  A^T = NUM^T * (1/Z broadcast via small replication matmul)
  out^T[dm, q] = Wo^T-matmul(A^T) + bo'  (bo' = Wo@bv + bo host-folded,
     valid because attention weights sum to 1)
  out = PE-transpose(out^T) -> DMA.
"""

import numpy as np

import concourse.bass as bass
import concourse.bacc as bacc
import concourse.mybir as mybir
import concourse.tile as tile
from concourse.bass_utils import run_bass_kernel_spmd
from concourse.masks import make_identity

B, N, D, H, DH = 4, 1024, 256, 8, 32
NB = 16
MAX_Z = 5.0
SCALE = DH ** (-0.5)
NCORES = 8
QR = N // 2  # query rows per core
P = 128
F32 = mybir.dt.float32
F16 = mybir.dt.float16

_CACHE = {}


def _build(z_emb: np.ndarray):
    """Build the (core-uniform) Bass program; z_emb baked as immediates."""
    nc = bacc.Bacc(trn_type="TRN2")

    c0 = z_emb[0, :].astype(np.float64)          # [H]
    dval = (z_emb[1:, :] - z_emb[:-1, :]).astype(np.float64)  # [15, H]

    xT = nc.dram_tensor("xT", [D, N], F32, kind="ExternalInput")
    xTq = nc.dram_tensor("xTq", [D, QR], F32, kind="ExternalInput")
    zT = nc.dram_tensor("zT", [N, QR], F32, kind="ExternalInput")
    wqT = nc.dram_tensor("wqT", [D, D], F32, kind="ExternalInput")
    wkT = nc.dram_tensor("wkT", [D, D], F32, kind="ExternalInput")
    wvT = nc.dram_tensor("wvT", [D, D], F32, kind="ExternalInput")
    woT = nc.dram_tensor("woT", [D, D], F32, kind="ExternalInput")
    kmadd = nc.dram_tensor("kmadd", [N, 1], F32, kind="ExternalInput")
    boT = nc.dram_tensor("boT", [D, 1], F32, kind="ExternalInput")
    out = nc.dram_tensor("out", [QR, D], F32, kind="ExternalOutput")

    NKC = N // P   # 8 key chunks
    NDC = D // P   # 2 d_model chunks

    with tile.TileContext(nc) as tc:
        with (
            tc.tile_pool(name="const", bufs=1) as const,
            tc.tile_pool(name="win", bufs=1) as win,
            tc.tile_pool(name="acts", bufs=1) as acts,
            tc.tile_pool(name="zpool", bufs=2) as zpool,
            tc.tile_pool(name="masks", bufs=1) as maskp,
            tc.tile_pool(name="diag", bufs=3) as diagp,
            tc.tile_pool(name="epool", bufs=6) as epool,
            tc.tile_pool(name="misc", bufs=1) as misc,
            tc.tile_pool(name="outp", bufs=1) as outp,
            # PSUM budget: psc 4 tags x 1 buf + pnum 3 tags + pmisc 1 = 8
            tc.tile_pool(name="psc", bufs=1, space="PSUM") as psc,
            tc.tile_pool(name="pnum", bufs=1, space="PSUM") as pnum,
            tc.tile_pool(name="pmisc", bufs=1, space="PSUM") as pmisc,
        ):
            # ---------------- constants ----------------
            ident16 = const.tile([P, P], F16, tag="i16", name="i16")
            make_identity(nc, ident16[:])
            ident32 = const.tile([P, P], F32, tag="i32", name="i32")
            make_identity(nc, ident32[:])
            # ones row for partition-broadcast matmuls
            ones32 = const.tile([1, 32], F32, tag="ones32", name="ones32")
            nc.gpsimd.memset(ones32[:], 1.0)

            # ---------------- input DMAs ----------------
            xT_sb, xTq_sb = [], []
            for c in range(NDC):
                t = win.tile([P, N], F32, tag=f"xt{c}", name=f"xt{c}")
                nc.sync.dma_start(t[:], xT[c * P:(c + 1) * P, :])
                xT_sb.append(t)
                t = win.tile([P, QR], F32, tag=f"xtq{c}", name=f"xtq{c}")
                nc.sync.dma_start(t[:], xTq[c * P:(c + 1) * P, :])
                xTq_sb.append(t)
            w_sb = {}
            for name, dram in (("q", wqT), ("k", wkT), ("v", wvT), ("o", woT)):
                for c in range(NDC):
                    t = win.tile([P, D], F32, tag=f"w{name}{c}", name=f"w{name}{c}")
                    nc.sync.dma_start(t[:], dram[c * P:(c + 1) * P, :])
                    w_sb[name, c] = t
            km_sb = []
            for kc in range(NKC):
                t = win.tile([P, 1], F32, tag=f"km{kc}", name=f"km{kc}")
                nc.sync.dma_start(t[:], kmadd[kc * P:(kc + 1) * P, :])
                km_sb.append(t)
            boT_sb = []
            for c in range(NDC):
                t = win.tile([P, 1], F32, tag=f"bo{c}", name=f"bo{c}")
                nc.sync.dma_start(t[:], boT[c * P:(c + 1) * P, :])
                boT_sb.append(t)

            # ---------------- projections ----------------
            # per-head tiles so matmul operands sit at base partition 0
            KT_sb = [acts.tile([DH, N], F32, tag=f"kth{h}", name=f"kth{h}") for h in range(H)]
            QT_sb = [acts.tile([DH, QR], F32, tag=f"qth{h}", name=f"qth{h}") for h in range(H)]
            for hc in range(NDC):
                for nb in range(N // 512):
                    ps = pmisc.tile([P, 512], F32, tag="pm", name="pm")
                    for dc in range(NDC):
                        nc.tensor.matmul(
                            ps[:],
                            w_sb["k", dc][:, hc * P:(hc + 1) * P],
                            xT_sb[dc][:, nb * 512:(nb + 1) * 512],
                            start=(dc == 0), stop=(dc == NDC - 1),
                        )
                    for hr in range(4):
                        nc.scalar.copy(
                            KT_sb[4 * hc + hr][:, nb * 512:(nb + 1) * 512],
                            ps[32 * hr:32 * hr + 32, :],
                        )
                ps = pmisc.tile([P, QR], F32, tag="pm", name="pm")
                for dc in range(NDC):
                    nc.tensor.matmul(
                        ps[:],
                        w_sb["q", dc][:, hc * P:(hc + 1) * P],
                        xTq_sb[dc][:],
                        start=(dc == 0), stop=(dc == NDC - 1),
                    )
                for hr in range(4):
                    nc.scalar.copy(
                        QT_sb[4 * hc + hr][:], ps[32 * hr:32 * hr + 32, :]
                    )

            # V_aug[k, 33h+d] fp16, col 33h+32 = ones
            V_sb = [acts.tile([P, 33 * H], F16, tag=f"v{kc}", name=f"v{kc}") for kc in range(NKC)]
            for kc in range(NKC):
                ps = pmisc.tile([P, D], F32, tag="pm", name="pm")
                for dc in range(NDC):
                    nc.tensor.matmul(
                        ps[:],
                        xT_sb[dc][:, kc * P:(kc + 1) * P],
                        w_sb["v", dc][:],
                        start=(dc == 0), stop=(dc == NDC - 1),
                    )
                v3 = V_sb[kc][:].rearrange("p (h x) -> p h x", x=33)
                nc.scalar.copy(
                    v3[:, :, 0:32], ps[:].rearrange("p (h d) -> p h d", d=DH)
                )
                nc.vector.memset(v3[:, :, 32:33], 1.0)

            # NUM psum: 4 banks, 2 heads per bank at row offsets 0/64
            # (PE psum writes must start at a 32-aligned partition)
            num_ps = [pnum.tile([P, QR], F32, tag=f"num{j}", name=f"num{j}") for j in range(4)]

            def num_slice(h, rows):
                j, i = divmod(h, 2)
                return num_ps[j][64 * i: 64 * i + rows, :]

            # exp bias tiles: kmadd_chunk + z_emb[0, h]
            cb = {}
            for h in range(H):
                for kc in range(NKC):
                    t = win.tile([P, 1], F32, tag=f"cb{h}_{kc}", name=f"cb{h}_{kc}")
                    nc.vector.tensor_scalar_add(t[:], km_sb[kc][:], float(c0[h]))
                    cb[h, kc] = t

            # ---------------- main loop: groups of key chunks ------------
            # 3 score psum banks + 4 NUM banks + 1 misc = 8
            for kcs in ([0, 1, 2], [3, 4, 5], [6, 7]):
                # threshold masks for these 4 key chunks
                mk = {}
                for gi, kc in enumerate(kcs):
                    zt = zpool.tile([P, QR], F32, tag="zt", name="zt")
                    nc.sync.dma_start(zt[:], zT[kc * P:(kc + 1) * P, :])
                    u = zpool.tile([P, QR], F32, tag="u", name="u")
                    nc.vector.tensor_scalar(
                        u[:], zt[:], float(NB / MAX_Z), None,
                        op0=mybir.AluOpType.mult,
                    )
                    for t_ in range(1, NB):
                        m = maskp.tile([P, QR], F16, tag=f"mk{gi}_{t_}", name=f"mk{gi}_{t_}")
                        nc.vector.tensor_scalar(
                            m[:], u[:], float(t_), None,
                            op0=mybir.AluOpType.is_ge,
                        )
                        mk[kc, t_] = m

                # per head: scores + bias -> exp -> NUM accumulate
                for h in range(H):
                    sc = {}
                    for gi, kc in enumerate(kcs):
                        ps = psc.tile([P, QR], F32, tag=f"sc{gi}", name=f"sc{gi}")
                        nc.tensor.matmul(
                            ps[:],
                            KT_sb[h][:, kc * P:(kc + 1) * P],
                            QT_sb[h][:],
                            start=True, stop=False,
                        )
                        sc[kc] = ps
                    for t_ in range(1, NB):
                        dg = diagp.tile([P, P], F16, tag="dg", name="dg")
                        nc.vector.tensor_scalar(
                            dg[:], ident16[:], float(dval[t_ - 1, h] / SCALE),
                            None, op0=mybir.AluOpType.mult,
                        )
                        for kc in kcs:
                            nc.tensor.matmul(
                                sc[kc][:], dg[:], mk[kc, t_][:],
                                start=False, stop=(t_ == NB - 1),
                            )
                    for kc in kcs:
                        e = epool.tile([P, QR], F16, tag="e", name="e")
                        nc.scalar.activation(
                            e[:], sc[kc][:], mybir.ActivationFunctionType.Exp,
                            bias=cb[h, kc][:], scale=float(SCALE),
                        )
                        nc.tensor.matmul(
                            num_slice(h, 33),
                            V_sb[kc][:, 33 * h: 33 * h + 33],
                            e[:],
                            start=(kc == 0), stop=(kc == NKC - 1),
                        )

            # ---------------- normalize + out-projection ----------------
            An = [outp.tile([P, QR], F32, tag=f"an{c}", name=f"an{c}") for c in range(NDC)]
            for h in range(H):
                hc, hr = divmod(h, 4)
                rsl = slice(32 * hr, 32 * hr + 32)
                zr = misc.tile([1, QR], F32, tag="zr", name="zr")
                nc.vector.tensor_scalar_add(
                    zr[:], num_slice(h, 33)[32:33, :], 1e-30
                )
                zrinv = misc.tile([1, QR], F32, tag="zrinv", name="zrinv")
                nc.vector.reciprocal(zrinv[:], zr[:])
                rp = pmisc.tile([32, QR], F32, tag="pm", name="pm")
                nc.tensor.matmul(rp[:], ones32[:], zrinv[:], start=True, stop=True)
                rp_sb = misc.tile([32, QR], F32, tag="rp_sb", name="rp_sb")
                nc.scalar.copy(rp_sb[:], rp[:])
                nc.vector.tensor_tensor(
                    An[hc][rsl, :], num_slice(h, 32), rp_sb[:],
                    op=mybir.AluOpType.mult,
                )

            oT = []
            for mc in range(NDC):
                ps = pmisc.tile([P, QR], F32, tag="pm", name="pm")
                for cc in range(NDC):
                    nc.tensor.matmul(
                        ps[:],
                        w_sb["o", cc][:, mc * P:(mc + 1) * P],
                        An[cc][:],
                        start=(cc == 0), stop=(cc == NDC - 1),
                    )
                ot = outp.tile([P, QR], F32, tag=f"ot{mc}", name=f"ot{mc}")
                nc.scalar.add(ot[:], ps[:], boT_sb[mc][:])
                oT.append(ot)

            # transpose out^T [dm, q] -> out [q, dm] and DMA
            for qb in range(QR // P):
                osb = outp.tile([P, D], F32, tag="osb", name="osb")
                for mc in range(NDC):
                    tp = pmisc.tile([P, P], F32, tag="pm", name="pm")
                    nc.tensor.transpose(
                        tp[:], oT[mc][:, qb * P:(qb + 1) * P], ident32[:]
                    )
                    nc.scalar.copy(osb[:, mc * P:(mc + 1) * P], tp[:])
                nc.sync.dma_start(out[qb * P:(qb + 1) * P, :], osb[:])

    if not nc.is_finalized():
        nc.finalize()
    return nc


def _prep_inputs(x, z_matrix, key_mask, Wq, bq, Wk, bk, Wv, bv, Wo, bo):
    f32 = np.float32
    assert np.all(np.asarray(bq) == 0) and np.all(np.asarray(bk) == 0), (
        "nonzero bq/bk not supported by this kernel build"
    )
    wqT = np.ascontiguousarray(np.asarray(Wq).T.astype(f32))
    wkT = np.ascontiguousarray(np.asarray(Wk).T.astype(f32))
    wvT = np.ascontiguousarray(np.asarray(Wv).T.astype(f32))
    woT = np.ascontiguousarray(np.asarray(Wo).T.astype(f32))
    # attention weights sum to 1 -> bv folds into output bias exactly
    bo_eff = (np.asarray(Wo) @ np.asarray(bv) + np.asarray(bo)).astype(f32)
    boT = np.ascontiguousarray(bo_eff.reshape(D, 1))

    in_maps = []
    for c in range(NCORES):
        b, half = divmod(c, 2)
        q0 = half * QR
        xb = np.asarray(x[b], dtype=f32)                    # [N, D]
        xT_ = np.ascontiguousarray(xb.T)                    # [D, N]
        xTq_ = np.ascontiguousarray(xb[q0:q0 + QR, :].T)    # [D, QR]
        zT_ = np.ascontiguousarray(
            np.asarray(z_matrix[b], dtype=f32).T[:, q0:q0 + QR]
        )                                                   # [N, QR]
        kma = np.ascontiguousarray(
            (np.asarray(key_mask[b]).astype(f32) * -1e30).reshape(N, 1)
        )
        in_maps.append({
            "xT": xT_, "xTq": xTq_, "zT": zT_,
            "wqT": wqT, "wkT": wkT, "wvT": wvT, "woT": woT,
            "kmadd": kma, "boT": boT,
        })
    return in_maps


def kernel(**inputs) -> np.ndarray:
    z_emb = np.asarray(inputs["z_emb"], dtype=np.float32)
    key = z_emb.tobytes()
    if key not in _CACHE:
        _CACHE[key] = _build(z_emb)
    nc = _CACHE[key]

    in_maps = _prep_inputs(
        inputs["x"], inputs["z_matrix"], inputs["key_mask"],
        inputs["Wq"], inputs["bq"], inputs["Wk"], inputs["bk"],
        inputs["Wv"], inputs["bv"], inputs["Wo"], inputs["bo"],
    )
    res = run_bass_kernel_spmd(nc, in_maps, core_ids=list(range(NCORES)))
    full = np.empty((B, N, D), dtype=np.float32)
    for c in range(NCORES):
        b, half = divmod(c, 2)
        full[b, half * QR:(half + 1) * QR, :] = res.results[c]["out"]
    return full

